# revision 8
# baseline (speedup 1.0000x reference)
"""Causal self-attention (b=2, t=2048, n_embd=768, n_head=12) on 8 TRN2 cores.

Sharding: core c -> batch b = c // 4, head group g = c % 4 (3 heads per group).
Each core computes, for its batch slice x_b [2048, 768] and its 3 heads:
  qkv slice -> per-head causal attention -> partial projection y_part [2048, 768]
using w_proj rows for those heads.  Host sums the 4 partial outputs per batch.

On-core layout (everything "transposed", head_dim on partitions):
  xT   [128e, 6, 2048t]   via PE transposes of x tiles
  k01  [128, 2048] = [kT_h0 ; kT_h1]   (partitions 0-63 / 64-127)
  q01  [128, 2048] = [qT_h0 ; qT_h1]
  kq2  [128, 2048] = [kT_h2 ; qT_h2], qT_h2 re-homed to base 0 via DMA
  v_aug[h] [128k, 16, 65]  v natural + ones column (softmax denominator trick)
  scores sT[k,q] = kT.T@qT per 128k x 512q tile, exp on ACT, causal mask by
  multiplying precomputed 0/1 diagonal masks, att@v accumulated in PSUM as
  outT [64d, 512q] with row 64 = denominator.  Normalize via PE outer-product
  broadcast of 1/S.  Proj contracts outT over d (heads 0,1 packed to K=128).

All matmuls run in float32r (TF32-like, 1 cycle/row at N>=256, ~1e-4 rel err).
"""

import os
import numpy as np
from contextlib import ExitStack

import concourse.bass as bass
import concourse.mybir as mybir
import concourse.tile as tile
from concourse import bacc
from concourse.bass_utils import run_bass_kernel_spmd
from concourse.masks import make_identity

F32 = mybir.dt.float32
F32R = mybir.dt.float32r
AF = mybir.ActivationFunctionType

T = 2048
E = 768
D = 64  # head dim
EC = E // 128  # 6 e-chunks
TC = T // 128  # 16 t-chunks
NS = T // 512  # 4 q-strips
SCALE = 1.0 / 8.0  # 1/sqrt(64)

_CACHED = {}


def build_nc():
    nc = bacc.Bacc("TRN2", target_bir_lowering=False, debug=False)
    x_d = nc.declare_dram_parameter("x", [T, E], F32, isOutput=False)
    wqk_d = nc.declare_dram_parameter("wqk", [E, 384], F32, isOutput=False)
    wv_d = nc.declare_dram_parameter("wv", [E, 256], F32, isOutput=False)
    wp_d = nc.declare_dram_parameter("wp", [192, E], F32, isOutput=False)
    y_d = nc.declare_dram_parameter("y", [T, E], F32, isOutput=True)

    with tile.TileContext(nc) as tc, ExitStack() as ctx:
        singles = ctx.enter_context(tc.tile_pool(name="singles", bufs=1))
        pool_x = ctx.enter_context(tc.tile_pool(name="xnat", bufs=4))
        pool_exp = ctx.enter_context(tc.tile_pool(name="exp", bufs=3))
        pool_tmp = ctx.enter_context(tc.tile_pool(name="tmp", bufs=2))
        pool_y = ctx.enter_context(tc.tile_pool(name="yout", bufs=3))
        ps_main = ctx.enter_context(tc.tile_pool(name="psmain", bufs=3, space="PSUM"))
        ps_acc = ctx.enter_context(tc.tile_pool(name="psacc", bufs=2, space="PSUM"))
        ps_bc = ctx.enter_context(tc.tile_pool(name="psbc", bufs=1, space="PSUM"))
        ps_proj = ctx.enter_context(tc.tile_pool(name="psproj", bufs=2, space="PSUM"))

        # ---- constants ----
        ident = singles.tile([128, 128], F32, tag="ident")
        make_identity(nc, ident)

        masks = []
        for j in range(4):
            m = singles.tile([128, 512], F32, tag=f"mask{j}")
            nc.gpsimd.memset(m, 1.0)
            # keep where (q_off) - (k_off) - j*128 >= 0, else 0
            nc.gpsimd.affine_select(
                out=m,
                in_=m,
                compare_op=mybir.AluOpType.is_ge,
                fill=0.0,
                base=-(j * 128),
                pattern=[[1, 512]],
                channel_multiplier=-1,
            )
            masks.append(m)

        ones_f32 = singles.tile([1, 64], F32, tag="ones_f32")
        nc.vector.memset(ones_f32, 1.0)
        ones_r = singles.tile([1, 64], F32R, tag="ones_r")
        nc.vector.tensor_copy(ones_r[:], ones_f32[:])

        # ---- weights ----
        wqk_sb = singles.tile([128, EC, 384], F32, tag="wqk_sb")
        nc.sync.dma_start(wqk_sb[:], wqk_d.rearrange("(eo p) c -> p eo c", p=128))
        wqk_r = singles.tile([128, EC, 384], F32R, tag="wqk_r")
        nc.vector.tensor_copy(wqk_r[:], wqk_sb[:])

        wv_sb = singles.tile([128, EC, 256], F32, tag="wv_sb")
        nc.sync.dma_start(wv_sb[:], wv_d.rearrange("(eo p) c -> p eo c", p=128))
        wv_r = singles.tile([128, EC, 256], F32R, tag="wv_r")
        nc.vector.tensor_copy(wv_r[:], wv_sb[:])

        wp01_sb = singles.tile([128, E], F32, tag="wp01_sb")
        nc.sync.dma_start(wp01_sb[:], wp_d[0:128, :])
        wp01_r = singles.tile([128, E], F32R, tag="wp01_r")
        nc.vector.tensor_copy(wp01_r[:], wp01_sb[:])
        wp2_sb = singles.tile([64, E], F32, tag="wp2_sb")
        nc.sync.dma_start(wp2_sb[:], wp_d[128:192, :])
        wp2_r = singles.tile([64, E], F32R, tag="wp2_r")
        nc.vector.tensor_copy(wp2_r[:], wp2_sb[:])

        # ---- persistent intermediates ----
        xT = singles.tile([128, EC, T], F32R, tag="xT")
        k01 = singles.tile([128, T], F32R, tag="k01")
        q01 = singles.tile([128, T], F32R, tag="q01")
        kq2 = singles.tile([128, T], F32R, tag="kq2")
        qt2_r = singles.tile([64, T], F32R, tag="qt2_r")
        va = [
            singles.tile([128, TC, 65], F32R, tag=f"va{h}", name=f"va{h}")
            for h in range(3)
        ]
        outT01 = singles.tile([128, T], F32R, tag="outT01")
        outT2 = singles.tile([64, T], F32R, tag="outT2")

        # ones column of v_aug (denominator accumulator weights)
        ones_col = singles.tile([128, TC], F32, tag="ones_col")
        nc.vector.memset(ones_col, 1.0)
        for h in range(3):
            nc.vector.tensor_copy(va[h][:, :, 64], ones_col[:])

        # ---- phase A: load x, transpose to xT ----
        for t_i in range(TC):
            x_sb = pool_x.tile([128, E], F32, tag="x")
            nc.sync.dma_start(x_sb[:], x_d[t_i * 128 : (t_i + 1) * 128, :])
            for ec in range(EC):
                pt = ps_main.tile([128, 512], F32, tag="mm", name="pt")
                nc.tensor.transpose(pt[:, 0:128], x_sb[:, ec * 128 : (ec + 1) * 128], ident[:])
                nc.vector.tensor_copy(xT[:, ec, t_i * 128 : (t_i + 1) * 128], pt[:, 0:128])

        # ---- phase B: qkT = wqk.T @ xT ----
        qkT = [k01, q01, kq2]
        for cc in range(3):
            for s in range(NS):
                pq = ps_main.tile([128, 512], F32, tag="mm", name="pq")
                for ec in range(EC):
                    nc.tensor.matmul(
                        pq[:],
                        wqk_r[:, ec, cc * 128 : (cc + 1) * 128],
                        xT[:, ec, s * 512 : (s + 1) * 512],
                        start=(ec == 0),
                        stop=(ec == EC - 1),
                    )
                nc.vector.tensor_copy(qkT[cc][:, s * 512 : (s + 1) * 512], pq[:])
                if cc == 2:
                    # re-home qT_h2 (partitions 64:128) to base 0 via SBUF->SBUF DMA
                    nc.sync.dma_start(
                        qt2_r[0:64, s * 512 : (s + 1) * 512],
                        kq2[64:128, s * 512 : (s + 1) * 512],
                    )

        # ---- phase C: v natural = x @ wv ----
        for t_i in range(TC):
            pv = ps_main.tile([128, 512], F32, tag="mm", name="pv")
            for ec in range(EC):
                nc.tensor.matmul(
                    pv[:, 0:256],
                    xT[:, ec, t_i * 128 : (t_i + 1) * 128],
                    wv_r[:, ec, :],
                    start=(ec == 0),
                    stop=(ec == EC - 1),
                )
            for h in range(3):
                nc.vector.tensor_copy(
                    va[h][:, t_i, 0:64], pv[:, h * 64 : (h + 1) * 64]
                )

        # ---- phase D: attention per (strip, head) ----
        for s in range(NS):
            nchunks = 4 * (s + 1)
            for h in range(3):
                acc = ps_acc.tile([128, 512], F32, tag="acc")
                for kc in range(nchunks):
                    pss = ps_main.tile([128, 512], F32, tag="mm", name="pss")
                    if h == 0:
                        lhs = k01[0:64, kc * 128 : (kc + 1) * 128]
                        rhs = q01[0:64, s * 512 : (s + 1) * 512]
                    elif h == 1:
                        lhs = k01[64:128, kc * 128 : (kc + 1) * 128]
                        rhs = q01[64:128, s * 512 : (s + 1) * 512]
                    else:
                        lhs = kq2[0:64, kc * 128 : (kc + 1) * 128]
                        rhs = qt2_r[0:64, s * 512 : (s + 1) * 512]
                    nc.tensor.matmul(pss[:], lhs, rhs, start=True, stop=True)
                    expT = pool_exp.tile([128, 512], F32R, tag="expT")
                    nc.scalar.activation(expT[:], pss[:], AF.Exp, scale=SCALE)
                    j = kc - 4 * s
                    if j >= 0:
                        nc.vector.tensor_mul(expT[:], expT[:], masks[j][:])
                    first = kc == 0
                    last = kc == nchunks - 1
                    nc.tensor.matmul(
                        acc[0:65],
                        va[h][:, kc, 0:65],
                        expT[:],
                        start=first,
                        stop=last,
                    )
                # normalize: outT = acc_out / S  (S in acc row 64)
                recip = pool_tmp.tile([1, 512], F32R, tag="recip")
                with nc.allow_low_precision(reason="1/S broadcast needs f32r"):
                    nc.vector.reciprocal(recip[:], acc[64:65, :])
                bc = ps_bc.tile([128, 512], F32, tag="bc")
                nc.tensor.matmul(
                    bc[0:64],
                    ones_r[:],
                    recip[:],
                    start=True,
                    stop=True,
                )
                tmp = pool_tmp.tile([128, 512], F32, tag="otmp")
                nc.scalar.copy(tmp[0:64, :], acc[0:64, :])
                if h == 0:
                    nc.vector.tensor_mul(
                        outT01[0:64, s * 512 : (s + 1) * 512], tmp[0:64, :], bc[0:64, :]
                    )
                elif h == 2:
                    nc.vector.tensor_mul(
                        outT2[0:64, s * 512 : (s + 1) * 512], tmp[0:64, :], bc[0:64, :]
                    )
                else:
                    # head 1 lands at partitions 64:128 of outT01 -> shift via DMA
                    stage = pool_tmp.tile([64, 512], F32R, tag="stage")
                    nc.vector.tensor_mul(stage[:], tmp[0:64, :], bc[0:64, :])
                    nc.sync.dma_start(
                        outT01[64:128, s * 512 : (s + 1) * 512], stage[:]
                    )

            # ---- phase E: projection for this strip's 4 q-chunks ----
            for qc in range(4):
                t_i = s * 4 + qc
                y_sb = pool_y.tile([128, E], F32, tag="y")
                for eh in range(2):
                    pp = ps_proj.tile([128, 512], F32, tag="pp", name="pp")[:, 0:384]
                    nc.tensor.matmul(
                        pp[:],
                        outT01[:, t_i * 128 : (t_i + 1) * 128],
                        wp01_r[:, eh * 384 : (eh + 1) * 384],
                        start=True,
                        stop=False,
                    )
                    nc.tensor.matmul(
                        pp[:],
                        outT2[0:64, t_i * 128 : (t_i + 1) * 128],
                        wp2_r[0:64, eh * 384 : (eh + 1) * 384],
                        start=False,
                        stop=True,
                    )
                    nc.vector.tensor_copy(y_sb[:, eh * 384 : (eh + 1) * 384], pp[:])
                nc.sync.dma_start(y_d[t_i * 128 : (t_i + 1) * 128, :], y_sb[:])

    nc.compile()
    return nc


def _shard_inputs(x, w_qkv, w_proj):
    in_maps = []
    for c in range(8):
        b, g = c // 4, c % 4
        h0 = 3 * g
        q = slice(h0 * D, (h0 + 2) * D)
        k = slice(E + h0 * D, E + (h0 + 2) * D)
        wqk = np.concatenate(
            [
                w_qkv[:, k],  # k_h0 | k_h1
                w_qkv[:, q],  # q_h0 | q_h1
                w_qkv[:, E + (h0 + 2) * D : E + (h0 + 3) * D],  # k_h2
                w_qkv[:, (h0 + 2) * D : (h0 + 3) * D],  # q_h2
            ],
            axis=1,
        )
        wv = np.concatenate(
            [
                w_qkv[:, 2 * E + h0 * D : 2 * E + (h0 + 3) * D],
                np.zeros((E, 64), dtype=np.float32),
            ],
            axis=1,
        )
        wp = w_proj[h0 * D : (h0 + 3) * D, :]
        in_maps.append(
            {
                "x": np.ascontiguousarray(x[b]),
                "wqk": np.ascontiguousarray(wqk),
                "wv": np.ascontiguousarray(wv),
                "wp": np.ascontiguousarray(wp),
            }
        )
    return in_maps


def kernel(x, w_qkv, w_proj):
    x = np.asarray(x, dtype=np.float32)
    w_qkv = np.asarray(w_qkv, dtype=np.float32)
    w_proj = np.asarray(w_proj, dtype=np.float32)

    if "nc" not in _CACHED:
        _CACHED["nc"] = build_nc()
    nc = _CACHED["nc"]

    in_maps = _shard_inputs(x, w_qkv, w_proj)
    trace = bool(int(os.environ.get("KERNEL_TRACE", "0")))
    res = run_bass_kernel_spmd(
        nc, in_maps, core_ids=list(range(8)), trace=trace
    )
    _CACHED["last_results"] = res

    y = np.zeros((2, T, E), dtype=np.float32)
    for c in range(8):
        y[c // 4] += res.results[c]["y"]
    return y


# revision 9
# speedup vs baseline: 1.0062x; 1.0062x over previous
"""Causal self-attention (b=2, t=2048, n_embd=768, n_head=12) on 8 TRN2 cores.

Sharding: core c -> batch b = c // 4, head group g = c % 4 (3 heads per group).
Each core computes, for its batch slice x_b [2048, 768] and its 3 heads:
  qkv slice -> per-head causal attention -> partial projection y_part [2048, 768]
using w_proj rows for those heads.  Host sums the 4 partial outputs per batch.

On-core layout (everything "transposed", head_dim on partitions):
  xT   [128e, 6, 2048t]   via PE transposes of x tiles
  k01  [128, 2048] = [kT_h0 ; kT_h1]   (partitions 0-63 / 64-127)
  q01  [128, 2048] = [qT_h0 ; qT_h1]
  kq2  [128, 2048] = [kT_h2 ; qT_h2], qT_h2 re-homed to base 0 via DMA
  va   [128k, 16, 3, 65]  v natural + ones column (softmax denominator trick)
  scores sT[k,q] = kT.T@qT per 128k x 512q tile (diagonal tiles trimmed to the
  valid column range), exp on ACT, causal triangular mask multiply on the
  128-wide diagonal block, att@v accumulated in PSUM as outT [64d, 512q] with
  row 64 = denominator.  The three heads' chunk streams are interleaved with
  att@v lagging one chunk so exp latency never stalls the PE.
  Normalize via PE outer-product broadcast of 1/S.  Proj contracts outT over
  d (heads 0,1 packed to K=128).

All matmuls run in float32r (TF32-like, 1 cycle/row at N>=256, ~1e-4 rel err).
"""

import os
import numpy as np
from contextlib import ExitStack

import concourse.bass as bass
import concourse.mybir as mybir
import concourse.tile as tile
from concourse import bacc
from concourse.bass_utils import run_bass_kernel_spmd
from concourse.masks import make_identity

F32 = mybir.dt.float32
F32R = mybir.dt.float32r
AF = mybir.ActivationFunctionType

T = 2048
E = 768
D = 64  # head dim
EC = E // 128  # 6 e-chunks
TC = T // 128  # 16 t-chunks
NS = T // 512  # 4 q-strips
SCALE = 1.0 / 8.0  # 1/sqrt(64)

_CACHED = {}


def build_nc():
    nc = bacc.Bacc("TRN2", target_bir_lowering=False, debug=False)
    x_d = nc.declare_dram_parameter("x", [T, E], F32, isOutput=False)
    wqk_d = nc.declare_dram_parameter("wqk", [E, 384], F32, isOutput=False)
    wv_d = nc.declare_dram_parameter("wv", [E, 256], F32, isOutput=False)
    wp_d = nc.declare_dram_parameter("wp", [192, E], F32, isOutput=False)
    y_d = nc.declare_dram_parameter("y", [T, E], F32, isOutput=True)

    with tile.TileContext(nc) as tc, ExitStack() as ctx:
        singles = ctx.enter_context(tc.tile_pool(name="singles", bufs=1))
        pool_x = ctx.enter_context(tc.tile_pool(name="xnat", bufs=4))
        pool_exp = ctx.enter_context(tc.tile_pool(name="exp", bufs=8))
        pool_tmp = ctx.enter_context(tc.tile_pool(name="tmp", bufs=3))
        pool_y = ctx.enter_context(tc.tile_pool(name="yout", bufs=3))
        # PSUM budget: mm 5 banks + acc 3 banks = 8
        ps_main = ctx.enter_context(tc.tile_pool(name="psmain", bufs=5, space="PSUM"))
        ps_acc = ctx.enter_context(tc.tile_pool(name="psacc", bufs=3, space="PSUM"))

        def mm_tile(name):
            return ps_main.tile([128, 512], F32, tag="mm", name=name)

        # ---- constants ----
        ident = singles.tile([128, 128], F32, tag="ident")
        make_identity(nc, ident)

        # triangular mask for the 128-wide diagonal block: keep f >= p
        trimask = singles.tile([128, 128], F32, tag="trimask")
        nc.gpsimd.memset(trimask, 1.0)
        nc.gpsimd.affine_select(
            out=trimask,
            in_=trimask,
            compare_op=mybir.AluOpType.is_ge,
            fill=0.0,
            base=0,
            pattern=[[1, 128]],
            channel_multiplier=-1,
        )

        ones_f32 = singles.tile([1, 64], F32, tag="ones_f32")
        nc.vector.memset(ones_f32, 1.0)
        ones_r = singles.tile([1, 64], F32R, tag="ones_r")
        nc.vector.tensor_copy(ones_r[:], ones_f32[:])

        # ---- weights ----
        wqk_sb = singles.tile([128, EC, 384], F32, tag="wqk_sb")
        nc.sync.dma_start(wqk_sb[:], wqk_d.rearrange("(eo p) c -> p eo c", p=128))
        wqk_r = singles.tile([128, EC, 384], F32R, tag="wqk_r")
        nc.vector.tensor_copy(wqk_r[:], wqk_sb[:])

        wv_sb = singles.tile([128, EC, 256], F32, tag="wv_sb")
        nc.sync.dma_start(wv_sb[:], wv_d.rearrange("(eo p) c -> p eo c", p=128))
        wv_r = singles.tile([128, EC, 256], F32R, tag="wv_r")
        nc.vector.tensor_copy(wv_r[:], wv_sb[:])

        wp01_sb = singles.tile([128, E], F32, tag="wp01_sb")
        nc.sync.dma_start(wp01_sb[:], wp_d[0:128, :])
        wp01_r = singles.tile([128, E], F32R, tag="wp01_r")
        nc.vector.tensor_copy(wp01_r[:], wp01_sb[:])
        wp2_sb = singles.tile([64, E], F32, tag="wp2_sb")
        nc.sync.dma_start(wp2_sb[:], wp_d[128:192, :])
        wp2_r = singles.tile([64, E], F32R, tag="wp2_r")
        nc.vector.tensor_copy(wp2_r[:], wp2_sb[:])

        # ---- persistent intermediates ----
        xT = singles.tile([128, EC, T], F32R, tag="xT")
        k01 = singles.tile([128, T], F32R, tag="k01")
        q01 = singles.tile([128, T], F32R, tag="q01")
        kq2 = singles.tile([128, T], F32R, tag="kq2")
        qt2_r = singles.tile([64, T], F32R, tag="qt2_r")
        va = singles.tile([128, TC, 3, 65], F32R, tag="va")
        outT01 = singles.tile([128, T], F32R, tag="outT01")
        outT2 = singles.tile([64, T], F32R, tag="outT2")

        # ones columns of va (denominator accumulator weights)
        ones_col = singles.tile([128, TC * 3], F32, tag="ones_col")
        nc.vector.memset(ones_col, 1.0)
        nc.vector.tensor_copy(va[:, :, :, 64], ones_col[:].rearrange("p (t h) -> p t h", h=3))

        # ---- phase A: load x, transpose to xT ----
        for t_i in range(TC):
            x_sb = pool_x.tile([128, E], F32, tag="x")
            nc.sync.dma_start(x_sb[:], x_d[t_i * 128 : (t_i + 1) * 128, :])
            for grp, ecs in ((0, (0, 1, 2, 3)), (1, (4, 5))):
                pt = mm_tile("pt")
                for i, ec in enumerate(ecs):
                    nc.tensor.transpose(
                        pt[:, i * 128 : (i + 1) * 128],
                        x_sb[:, ec * 128 : (ec + 1) * 128],
                        ident[:],
                    )
                w = 128 * len(ecs)
                nc.vector.tensor_copy(
                    xT[:, ecs[0] : ecs[0] + len(ecs), t_i * 128 : (t_i + 1) * 128],
                    pt[:, 0:w].rearrange("p (e t) -> p e t", t=128),
                )

        # ---- phases B+C interleaved per strip: qkT and v ----
        qkT = [k01, q01, kq2]
        for s in range(NS):
            for cc in range(3):
                pq = mm_tile("pq")
                for ec in range(EC):
                    nc.tensor.matmul(
                        pq[:],
                        wqk_r[:, ec, cc * 128 : (cc + 1) * 128],
                        xT[:, ec, s * 512 : (s + 1) * 512],
                        start=(ec == 0),
                        stop=(ec == EC - 1),
                    )
                nc.vector.tensor_copy(qkT[cc][:, s * 512 : (s + 1) * 512], pq[:])
                if cc == 2:
                    # re-home qT_h2 (partitions 64:128) to base 0 via SBUF->SBUF DMA
                    nc.sync.dma_start(
                        qt2_r[0:64, s * 512 : (s + 1) * 512],
                        kq2[64:128, s * 512 : (s + 1) * 512],
                    )
            for t_i in range(4 * s, 4 * s + 4):
                pv = mm_tile("pv")
                for ec in range(EC):
                    nc.tensor.matmul(
                        pv[:, 0:256],
                        xT[:, ec, t_i * 128 : (t_i + 1) * 128],
                        wv_r[:, ec, :],
                        start=(ec == 0),
                        stop=(ec == EC - 1),
                    )
                nc.vector.tensor_copy(
                    va[:, t_i, :, 0:64],
                    pv[:, 0:192].rearrange("p (h c) -> p h c", c=64),
                )

        # ---- phase D: attention, heads interleaved, att@v lags one chunk ----
        def qk_ap(h, kc, s, o):
            if h == 0:
                return (
                    k01[0:64, kc * 128 : (kc + 1) * 128],
                    q01[0:64, s * 512 + o : (s + 1) * 512],
                )
            if h == 1:
                return (
                    k01[64:128, kc * 128 : (kc + 1) * 128],
                    q01[64:128, s * 512 + o : (s + 1) * 512],
                )
            return (
                kq2[0:64, kc * 128 : (kc + 1) * 128],
                qt2_r[0:64, s * 512 + o : (s + 1) * 512],
            )

        for s in range(NS):
            n = 4 * (s + 1)
            accs = []
            for h in range(3):
                accs.append(ps_acc.tile([128, 512], F32, tag="acc", name=f"acc{s}{h}"))
            exps = {}

            def emit_scores(h, kc):
                j = kc - 4 * s
                o = 0 if j < 0 else j * 128
                pss = mm_tile("pss")
                lhs, rhs = qk_ap(h, kc, s, o)
                nc.tensor.matmul(pss[:, o:512], lhs, rhs, start=True, stop=True)
                expT = pool_exp.tile([128, 512], F32R, tag="expT", name="expT")
                nc.scalar.activation(expT[:, o:512], pss[:, o:512], AF.Exp, scale=SCALE)
                if j >= 0:
                    nc.vector.tensor_mul(
                        expT[:, o : o + 128], expT[:, o : o + 128], trimask[:]
                    )
                exps[(h, kc)] = expT

            def emit_attv(h, kc):
                j = kc - 4 * s
                o = 0 if j < 0 else j * 128
                expT = exps.pop((h, kc))
                nc.tensor.matmul(
                    accs[h][0:65, o:512],
                    va[:, kc, h, 0:65],
                    expT[:, o:512],
                    start=(kc == 0),
                    stop=(kc == n - 1),
                )

            for kc in range(n):
                for h in range(3):
                    emit_scores(h, kc)
                if kc > 0:
                    for h in range(3):
                        emit_attv(h, kc - 1)
            for h in range(3):
                emit_attv(h, n - 1)

            # normalize: outT = acc_out / S  (S in acc row 64)
            for h in range(3):
                acc = accs[h]
                recip = pool_tmp.tile([1, 512], F32R, tag="recip", name="recip")
                with nc.allow_low_precision(reason="1/S broadcast needs f32r"):
                    nc.vector.reciprocal(recip[:], acc[64:65, :])
                bc = mm_tile("bc")
                nc.tensor.matmul(bc[0:64], ones_r[:], recip[:], start=True, stop=True)
                tmp = pool_tmp.tile([128, 512], F32, tag="otmp", name="otmp")
                nc.scalar.copy(tmp[0:64, :], acc[0:64, :])
                if h == 0:
                    nc.vector.tensor_mul(
                        outT01[0:64, s * 512 : (s + 1) * 512], tmp[0:64, :], bc[0:64, :]
                    )
                elif h == 2:
                    nc.vector.tensor_mul(
                        outT2[0:64, s * 512 : (s + 1) * 512], tmp[0:64, :], bc[0:64, :]
                    )
                else:
                    # head 1 lands at partitions 64:128 of outT01 -> shift via DMA
                    stage = pool_tmp.tile([64, 512], F32R, tag="stage", name="stage")
                    nc.vector.tensor_mul(stage[:], tmp[0:64, :], bc[0:64, :])
                    nc.sync.dma_start(
                        outT01[64:128, s * 512 : (s + 1) * 512], stage[:]
                    )

            # ---- phase E: projection for this strip's 4 q-chunks ----
            for qc in range(4):
                t_i = s * 4 + qc
                y_sb = pool_y.tile([128, E], F32, tag="y")
                for eh in range(2):
                    pp = mm_tile("pp")
                    nc.tensor.matmul(
                        pp[:, 0:384],
                        outT01[:, t_i * 128 : (t_i + 1) * 128],
                        wp01_r[:, eh * 384 : (eh + 1) * 384],
                        start=True,
                        stop=False,
                    )
                    nc.tensor.matmul(
                        pp[:, 0:384],
                        outT2[0:64, t_i * 128 : (t_i + 1) * 128],
                        wp2_r[0:64, eh * 384 : (eh + 1) * 384],
                        start=False,
                        stop=True,
                    )
                    nc.vector.tensor_copy(
                        y_sb[:, eh * 384 : (eh + 1) * 384], pp[:, 0:384]
                    )
                nc.sync.dma_start(y_d[t_i * 128 : (t_i + 1) * 128, :], y_sb[:])

    nc.compile()
    return nc


def _shard_inputs(x, w_qkv, w_proj):
    in_maps = []
    for c in range(8):
        b, g = c // 4, c % 4
        h0 = 3 * g
        q = slice(h0 * D, (h0 + 2) * D)
        k = slice(E + h0 * D, E + (h0 + 2) * D)
        wqk = np.concatenate(
            [
                w_qkv[:, k],  # k_h0 | k_h1
                w_qkv[:, q],  # q_h0 | q_h1
                w_qkv[:, E + (h0 + 2) * D : E + (h0 + 3) * D],  # k_h2
                w_qkv[:, (h0 + 2) * D : (h0 + 3) * D],  # q_h2
            ],
            axis=1,
        )
        wv = np.concatenate(
            [
                w_qkv[:, 2 * E + h0 * D : 2 * E + (h0 + 3) * D],
                np.zeros((E, 64), dtype=np.float32),
            ],
            axis=1,
        )
        wp = w_proj[h0 * D : (h0 + 3) * D, :]
        in_maps.append(
            {
                "x": np.ascontiguousarray(x[b]),
                "wqk": np.ascontiguousarray(wqk),
                "wv": np.ascontiguousarray(wv),
                "wp": np.ascontiguousarray(wp),
            }
        )
    return in_maps


def kernel(x, w_qkv, w_proj):
    x = np.asarray(x, dtype=np.float32)
    w_qkv = np.asarray(w_qkv, dtype=np.float32)
    w_proj = np.asarray(w_proj, dtype=np.float32)

    if "nc" not in _CACHED:
        _CACHED["nc"] = build_nc()
    nc = _CACHED["nc"]

    in_maps = _shard_inputs(x, w_qkv, w_proj)
    trace = bool(int(os.environ.get("KERNEL_TRACE", "0")))
    res = run_bass_kernel_spmd(
        nc, in_maps, core_ids=list(range(8)), trace=trace
    )
    _CACHED["last_results"] = res

    y = np.zeros((2, T, E), dtype=np.float32)
    for c in range(8):
        y[c // 4] += res.results[c]["y"]
    return y


# revision 13
# speedup vs baseline: 1.0870x; 1.0802x over previous
"""Causal self-attention (b=2, t=2048, n_embd=768, n_head=12) on 8 TRN2 cores.

Sharding: core c -> batch b = c // 4, head group g = c % 4 (3 heads per group).
Each core computes, for its batch slice x_b [2048, 768] and its 3 heads:
  qkv slice -> per-head causal attention -> partial projection y_part [2048, 768]
using w_proj rows for those heads.  Host sums the 4 partial outputs per batch.

On-core layout (everything "transposed", head_dim on partitions):
  xT   [128e, 6, 2048t]   via PE transposes of x tiles
  k01  [128, 2048] = [kT_h0 ; kT_h1]   (partitions 0-63 / 64-127)
  q01  [128, 2048] = [qT_h0 ; qT_h1]
  kq2  [128, 2048] = [kT_h2 ; qT_h2], qT_h2 re-homed to base 0 via DMA
  va   [128k, 16, 3, 65]  v natural + ones column (softmax denominator trick)
  scores sT[k,q] = kT.T@qT per 128k x 512q tile (diagonal tiles trimmed to the
  valid column range), exp on ACT, causal triangular mask multiply on the
  128-wide diagonal block, att@v accumulated in PSUM as outT [64d, 512q] with
  row 64 = denominator.

The whole kernel runs strip-by-strip (A load/transpose -> B qkT -> C v ->
D attention -> E projection per 512-query strip) so the PE-heavy prep of strip
s+1 overlaps the ACT(exp)-bound attention of strip s.  Within attention the
three heads' chunk streams are interleaved and att@v lags the scores stream by
two chunks so exp latency never blocks the in-order PE queue.

All matmuls run in float32r (TF32-like, 1 cycle/row at N>=256, ~1e-4 rel err).
"""

import os
import numpy as np
from contextlib import ExitStack

import concourse.bass as bass
import concourse.mybir as mybir
import concourse.tile as tile
from concourse import bacc
from concourse.bass_utils import run_bass_kernel_spmd
from concourse.masks import make_identity

F32 = mybir.dt.float32
F32R = mybir.dt.float32r
AF = mybir.ActivationFunctionType

T = 2048
E = 768
D = 64  # head dim
EC = E // 128  # 6 e-chunks
TC = T // 128  # 16 t-chunks
NS = T // 512  # 4 q-strips
SCALE = 1.0 / 8.0  # 1/sqrt(64)
LAG = 2  # att@v chunk lag behind scores

_CACHED = {}


def build_nc():
    nc = bacc.Bacc("TRN2", target_bir_lowering=False, debug=False)
    x_d = nc.declare_dram_parameter("x", [T, E], F32, isOutput=False)
    wqk_d = nc.declare_dram_parameter("wqk", [E, 384], F32, isOutput=False)
    wv_d = nc.declare_dram_parameter("wv", [E, 256], F32, isOutput=False)
    wp_d = nc.declare_dram_parameter("wp", [192, E], F32, isOutput=False)
    y_d = nc.declare_dram_parameter("y", [T, E], F32, isOutput=True)

    with tile.TileContext(nc) as tc, ExitStack() as ctx:
        singles = ctx.enter_context(tc.tile_pool(name="singles", bufs=1))
        pool_x = ctx.enter_context(tc.tile_pool(name="xnat", bufs=3))
        pool_exp = ctx.enter_context(tc.tile_pool(name="exp", bufs=9))
        pool_tmp = ctx.enter_context(tc.tile_pool(name="tmp", bufs=3))
        pool_y = ctx.enter_context(tc.tile_pool(name="yout", bufs=3))
        # PSUM budget: mm 5 banks + acc 3 banks = 8
        ps_main = ctx.enter_context(tc.tile_pool(name="psmain", bufs=5, space="PSUM"))
        ps_acc = ctx.enter_context(tc.tile_pool(name="psacc", bufs=3, space="PSUM"))

        def mm_tile(name):
            return ps_main.tile([128, 512], F32, tag="mm", name=name)

        # ---- constants ----
        ident = singles.tile([128, 128], F32, tag="ident")
        make_identity(nc, ident)

        # triangular mask for the 128-wide diagonal block: keep f >= p
        trimask = singles.tile([128, 128], F32, tag="trimask")
        nc.gpsimd.memset(trimask, 1.0)
        nc.gpsimd.affine_select(
            out=trimask,
            in_=trimask,
            compare_op=mybir.AluOpType.is_ge,
            fill=0.0,
            base=0,
            pattern=[[1, 128]],
            channel_multiplier=-1,
        )

        ones_f32 = singles.tile([1, 64], F32, tag="ones_f32")
        nc.vector.memset(ones_f32, 1.0)
        ones_r = singles.tile([1, 64], F32R, tag="ones_r")
        nc.vector.tensor_copy(ones_r[:], ones_f32[:])

        # ---- weights (one shared fp32 staging tile, rounded into f32r tiles) ----
        stage_w = singles.tile([128, 2304], F32, tag="stage_w")
        wqk_r = singles.tile([128, EC, 384], F32R, tag="wqk_r")
        wv_r = singles.tile([128, EC, 256], F32R, tag="wv_r")
        wp01_r = singles.tile([128, E], F32R, tag="wp01_r")
        wp2_r = singles.tile([64, E], F32R, tag="wp2_r")

        v_qk = stage_w[:, 0:2304].rearrange("p (a b) -> p a b", b=384)
        nc.sync.dma_start(v_qk, wqk_d.rearrange("(eo p) c -> p eo c", p=128))
        nc.vector.tensor_copy(wqk_r[:], v_qk)

        v_v = stage_w[:, 0:1536].rearrange("p (a b) -> p a b", b=256)
        nc.sync.dma_start(v_v, wv_d.rearrange("(eo p) c -> p eo c", p=128))
        nc.vector.tensor_copy(wv_r[:], v_v)

        nc.sync.dma_start(stage_w[:, 0:E], wp_d[0:128, :])
        nc.vector.tensor_copy(wp01_r[:], stage_w[:, 0:E])
        nc.sync.dma_start(stage_w[0:64, E : 2 * E], wp_d[128:192, :])
        nc.vector.tensor_copy(wp2_r[:], stage_w[0:64, E : 2 * E])

        # ---- persistent intermediates ----
        xT = singles.tile([128, EC, T], F32R, tag="xT")
        k01 = singles.tile([128, T], F32R, tag="k01")
        q01 = singles.tile([128, T], F32R, tag="q01")
        kq2 = singles.tile([128, T], F32R, tag="kq2")
        qt2_r = singles.tile([64, T], F32R, tag="qt2_r")
        va = singles.tile([128, TC, 3, 65], F32R, tag="va")
        outT01 = singles.tile([128, T], F32R, tag="outT01")
        outT2 = singles.tile([64, T], F32R, tag="outT2")

        # ones columns of va (denominator accumulator weights)
        ones_col = singles.tile([128, TC * 3], F32, tag="ones_col")
        nc.vector.memset(ones_col, 1.0)
        nc.vector.tensor_copy(
            va[:, :, :, 64], ones_col[:].rearrange("p (t h) -> p t h", h=3)
        )

        qkT = [k01, q01, kq2]

        def qk_ap(h, kc, s, o):
            if h == 0:
                return (
                    k01[0:64, kc * 128 : (kc + 1) * 128],
                    q01[0:64, s * 512 + o : (s + 1) * 512],
                )
            if h == 1:
                return (
                    k01[64:128, kc * 128 : (kc + 1) * 128],
                    q01[64:128, s * 512 + o : (s + 1) * 512],
                )
            return (
                kq2[0:64, kc * 128 : (kc + 1) * 128],
                qt2_r[0:64, s * 512 + o : (s + 1) * 512],
            )

        for s in range(NS):
            # ---- A: load + transpose this strip's 4 x chunks ----
            for t_i in range(4 * s, 4 * s + 4):
                x_sb = pool_x.tile([128, E], F32, tag="x", name="x_sb")
                nc.sync.dma_start(x_sb[:], x_d[t_i * 128 : (t_i + 1) * 128, :])
                for ecs in ((0, 1, 2, 3), (4, 5)):
                    pt = mm_tile("pt")
                    for i, ec in enumerate(ecs):
                        nc.tensor.transpose(
                            pt[:, i * 128 : (i + 1) * 128],
                            x_sb[:, ec * 128 : (ec + 1) * 128],
                            ident[:],
                        )
                    w = 128 * len(ecs)
                    nc.vector.tensor_copy(
                        xT[:, ecs[0] : ecs[0] + len(ecs), t_i * 128 : (t_i + 1) * 128],
                        pt[:, 0:w].rearrange("p (e t) -> p e t", t=128),
                    )

            # ---- B: qkT for this strip ----
            for cc in range(3):
                pq = mm_tile("pq")
                for ec in range(EC):
                    nc.tensor.matmul(
                        pq[:],
                        wqk_r[:, ec, cc * 128 : (cc + 1) * 128],
                        xT[:, ec, s * 512 : (s + 1) * 512],
                        start=(ec == 0),
                        stop=(ec == EC - 1),
                    )
                nc.vector.tensor_copy(qkT[cc][:, s * 512 : (s + 1) * 512], pq[:])
                if cc == 2:
                    # re-home qT_h2 (partitions 64:128) to base 0 via SBUF->SBUF DMA
                    nc.sync.dma_start(
                        qt2_r[0:64, s * 512 : (s + 1) * 512],
                        kq2[64:128, s * 512 : (s + 1) * 512],
                    )

            # ---- C: v for this strip's 4 t-chunks ----
            for t_i in range(4 * s, 4 * s + 4):
                pv = mm_tile("pv")
                for ec in range(EC):
                    nc.tensor.matmul(
                        pv[:, 0:256],
                        xT[:, ec, t_i * 128 : (t_i + 1) * 128],
                        wv_r[:, ec, :],
                        start=(ec == 0),
                        stop=(ec == EC - 1),
                    )
                nc.vector.tensor_copy(
                    va[:, t_i, :, 0:64],
                    pv[:, 0:192].rearrange("p (h c) -> p h c", c=64),
                )

            # ---- D: attention (heads interleaved, att@v lags LAG chunks) ----
            n = 4 * (s + 1)
            accs = [
                ps_acc.tile([128, 512], F32, tag="acc", name=f"acc{s}{h}")
                for h in range(3)
            ]
            exps = {}

            def emit_scores(h, kc, s=s):
                j = kc - 4 * s
                o = 0 if j < 0 else j * 128
                pss = mm_tile("pss")
                lhs, rhs = qk_ap(h, kc, s, o)
                nc.tensor.matmul(pss[:, o:512], lhs, rhs, start=True, stop=True)
                expT = pool_exp.tile([128, 512], F32R, tag="expT", name="expT")
                nc.scalar.activation(expT[:, o:512], pss[:, o:512], AF.Exp, scale=SCALE)
                if j >= 0:
                    nc.vector.tensor_mul(
                        expT[:, o : o + 128], expT[:, o : o + 128], trimask[:]
                    )
                exps[(h, kc)] = expT

            def emit_attv(h, kc, s=s, n=n, accs=accs):
                j = kc - 4 * s
                o = 0 if j < 0 else j * 128
                expT = exps.pop((h, kc))
                nc.tensor.matmul(
                    accs[h][0:65, o:512],
                    va[:, kc, h, 0:65],
                    expT[:, o:512],
                    start=(kc == 0),
                    stop=(kc == n - 1),
                )

            for kc in range(n):
                for h in range(3):
                    emit_scores(h, kc)
                if kc >= LAG:
                    for h in range(3):
                        emit_attv(h, kc - LAG)
            for kc in range(max(0, n - LAG), n):
                for h in range(3):
                    emit_attv(h, kc)

            # normalize: outT = acc_out / S  (S in acc row 64)
            for h in range(3):
                acc = accs[h]
                recip = pool_tmp.tile([1, 512], F32R, tag="recip", name="recip")
                with nc.allow_low_precision(reason="1/S broadcast needs f32r"):
                    nc.vector.reciprocal(recip[:], acc[64:65, :])
                bc = mm_tile("bc")
                nc.tensor.matmul(bc[0:64], ones_r[:], recip[:], start=True, stop=True)
                tmp = pool_tmp.tile([128, 512], F32, tag="otmp", name="otmp")
                nc.vector.tensor_copy(tmp[0:64, :], acc[0:64, :])
                if h == 0:
                    nc.vector.tensor_mul(
                        outT01[0:64, s * 512 : (s + 1) * 512], tmp[0:64, :], bc[0:64, :]
                    )
                elif h == 2:
                    nc.vector.tensor_mul(
                        outT2[0:64, s * 512 : (s + 1) * 512], tmp[0:64, :], bc[0:64, :]
                    )
                else:
                    # head 1 lands at partitions 64:128 of outT01 -> shift via DMA
                    stage = pool_tmp.tile([64, 512], F32R, tag="stage", name="stage")
                    nc.vector.tensor_mul(stage[:], tmp[0:64, :], bc[0:64, :])
                    nc.sync.dma_start(
                        outT01[64:128, s * 512 : (s + 1) * 512], stage[:]
                    )

            # ---- E: projection for this strip's 4 q-chunks ----
            for qc in range(4):
                t_i = s * 4 + qc
                y_sb = pool_y.tile([128, E], F32, tag="y", name="y_sb")
                for eh in range(2):
                    pp = mm_tile("pp")
                    nc.tensor.matmul(
                        pp[:, 0:384],
                        outT01[:, t_i * 128 : (t_i + 1) * 128],
                        wp01_r[:, eh * 384 : (eh + 1) * 384],
                        start=True,
                        stop=False,
                    )
                    nc.tensor.matmul(
                        pp[:, 0:384],
                        outT2[0:64, t_i * 128 : (t_i + 1) * 128],
                        wp2_r[0:64, eh * 384 : (eh + 1) * 384],
                        start=False,
                        stop=True,
                    )
                    nc.vector.tensor_copy(
                        y_sb[:, eh * 384 : (eh + 1) * 384], pp[:, 0:384]
                    )
                nc.sync.dma_start(y_d[t_i * 128 : (t_i + 1) * 128, :], y_sb[:])

    nc.compile()
    return nc


def _shard_inputs(x, w_qkv, w_proj):
    in_maps = []
    for c in range(8):
        b, g = c // 4, c % 4
        h0 = 3 * g
        q = slice(h0 * D, (h0 + 2) * D)
        k = slice(E + h0 * D, E + (h0 + 2) * D)
        wqk = np.concatenate(
            [
                w_qkv[:, k],  # k_h0 | k_h1
                w_qkv[:, q],  # q_h0 | q_h1
                w_qkv[:, E + (h0 + 2) * D : E + (h0 + 3) * D],  # k_h2
                w_qkv[:, (h0 + 2) * D : (h0 + 3) * D],  # q_h2
            ],
            axis=1,
        )
        wv = np.concatenate(
            [
                w_qkv[:, 2 * E + h0 * D : 2 * E + (h0 + 3) * D],
                np.zeros((E, 64), dtype=np.float32),
            ],
            axis=1,
        )
        wp = w_proj[h0 * D : (h0 + 3) * D, :]
        in_maps.append(
            {
                "x": np.ascontiguousarray(x[b]),
                "wqk": np.ascontiguousarray(wqk),
                "wv": np.ascontiguousarray(wv),
                "wp": np.ascontiguousarray(wp),
            }
        )
    return in_maps


def kernel(x, w_qkv, w_proj):
    x = np.asarray(x, dtype=np.float32)
    w_qkv = np.asarray(w_qkv, dtype=np.float32)
    w_proj = np.asarray(w_proj, dtype=np.float32)

    if "nc" not in _CACHED:
        _CACHED["nc"] = build_nc()
    nc = _CACHED["nc"]

    in_maps = _shard_inputs(x, w_qkv, w_proj)
    trace = bool(int(os.environ.get("KERNEL_TRACE", "0")))
    res = run_bass_kernel_spmd(
        nc, in_maps, core_ids=list(range(8)), trace=trace
    )
    _CACHED["last_results"] = res

    y = np.zeros((2, T, E), dtype=np.float32)
    for c in range(8):
        y[c // 4] += res.results[c]["y"]
    return y
